# revision 16
# baseline (speedup 1.0000x reference)
"""Trainium2 Bass kernel v4 — dual-engine exp + software-pipelined attention.

Multi-head attention (B=2, N=4096, D=768, H=12, d_head=64) on 8 NeuronCores.
Data-parallel over batch (4 cores per element), tensor-parallel over heads
(3 heads per core). Host sums the 4 partial outputs per batch element.

v4 changes over v3 (which ran ~730us, scalar-exp-bound at ~500us of EXP):
  * exp split across TWO engines: the Activation engine keeps 22/32 key
    chunks (table exp, scale=16/log2e); the DVE takes 10/32 via two custom
    uop instructions: y1 = deg-3 poly ~ 2^v (v = logits*SCALE*log2e/16,
    produced directly by the S matmul because the 1/16*log2e*SCALE factor
    is folded into the q columns of W_qkv on the host), then p = y1^16 by
    four squarings. Max rel err 4.4e-3 on the DVE share (HW-validated).
  * software pipelining: per step i the emission order is S(i+1), EXP(i+1),
    O(i) — the PE computes the next chunk's logits while the current exp
    runs, so the exp engines never wait on the in-order PE queue.
  * all dtype casts moved to the host (x, W_qkv, W_out DMA'd as fp16);
    phase A engine copies only fan PSUM projections out to SBUF fp16.
  * V stored as vAll[128, kc, head, 128] = [V_h | ones | zeros] so the
    per-token-chunk projection copy is a single strided tensor_copy and the
    PV stationary tile is a contiguous slice per head.
  * output projection packed into head-pair tiles: A01 [128=2 heads, N],
    A2b [65 = h2+bias, N] against wout01 [128, 768] / wout2b [65, 768]:
    4 matmuls per 128-token chunk (was 6), emitted as one burst every 3rd
    pipeline step.

Weight layout [768, 704]: [q01*a(128) | k01(128) | q2*a,k2(128) | k2,q2*a
(128) | v012(192)], a = SCALE*log2e/16. The duplicated q2/k2 columns let
both h2 operands land in partition rows 0..63 without a cross-partition
move (engine copies cannot shift partitions; DMA cannot cast fp32->fp16).
"""

import numpy as np

import concourse.bass as bass
import concourse.tile as tile
from concourse import mybir, bacc
from concourse import dve_ops, dve_spec
from concourse.bass_utils import run_bass_kernel_spmd
from concourse.dve_spec import Spec, Src0, C0, C1, C2, One, Bin, AluOp
from concourse.dve_uop import DveOpSpec

F32 = mybir.dt.float32
F16 = mybir.dt.float16
EXP = mybir.ActivationFunctionType.Exp

N_CORES = 8
B = 2
N = 4096
D = 768
H = 12
HD = 64
SCALE = HD ** -0.5
LOG2E = 1.4426950408889634
ALPHA = SCALE * LOG2E / 16.0     # folded into q columns host-side
ACT_SCALE = 16.0 / LOG2E         # activation-engine exp compensation
A1, A2, A3 = 0.6932766207788288, 0.24270536127172082, 0.05546776495557384
DC = D // 128       # 6 contraction chunks
QC = 1024           # query block
NQC = N // QC       # 4
NKC = N // 128      # 32 key chunks
NSEG = 4
SEG = N // NSEG
SEGC = SEG // 128   # token chunks per segment

TRACE = False
TRACE_ALL_CORES = False
LAST_RESULT = None

_nc_cache = None


def _register(name, spec):
    """Register a custom DVE op at runtime (same effect as adding it to
    dve_ops.OPS in-source); idempotent by name."""
    if name in dve_ops._SUB_OPCODE_FOR_NAME:
        return next(o for o in dve_ops.OPS if o.name == name)
    row = dve_ops._CUSTOM_DVE_ROW_BASE + len(dve_ops.OPS)
    assert row < 32, "custom-DVE row field is 5 bits"
    dve_ops._SUB_OPCODE_FOR_NAME[name] = row
    op = dve_ops.DveOp(name, spec, subdim=False, uops_sha={})
    shas = {}
    for ver in ("v3", "v4"):
        uops = dve_spec.lower(spec, ver=ver)
        s = DveOpSpec(name=name, opcode=row, uops=uops,
                      rd1_en=dve_spec._has_src1(spec))
        shas[ver] = s.sha(ver)
    object.__setattr__(op, "uops_sha", shas)
    dve_ops.OPS.append(op)
    dve_ops.CUSTOM_DVE_SPECS[name] = spec
    return op


# y = 1 + v*(a1 + v*(a2 + v*a3)) ~= 2^v on [-0.57, 0.57] (rel err 3e-4)
POLY_EXP2 = _register(
    "ANT_POLY_EXP2",
    Spec(
        body=One + Src0 * (C0 + Src0 * (C1 + Src0 * C2)),
        reference=lambda in0, in1, s0, s1, imm2:
            (1.0 + in0 * (s0 + in0 * (s1 + in0 * imm2))).astype(np.float32),
    ),
)

_sq = lambda x: Bin(AluOp.MULTIPLY, x, x)
POW16 = _register(
    "ANT_POW16",
    Spec(
        body=_sq(_sq(_sq(_sq(Src0)))),
        reference=lambda in0, in1, s0, s1, imm2:
            (in0.astype(np.float64) ** 16).astype(np.float32),
    ),
)


def _build_module():
    nc = bacc.Bacc("TRN2", target_bir_lowering=False, debug=False,
                   num_devices=N_CORES)
    x_d = nc.dram_tensor("x", [D, N], F16, kind="ExternalInput")
    wqkv_d = nc.dram_tensor("wqkv", [D, 704], F16, kind="ExternalInput")
    wout_d = nc.dram_tensor("wout", [193, D], F16, kind="ExternalInput")
    y_d = nc.dram_tensor("y", [N, D], F32, kind="ExternalOutput")

    with tile.TileContext(nc) as tc:
        _emit(nc, tc, x_d, wqkv_d, wout_d, y_d)
    nc.compile()
    return nc


def _emit(nc, tc, x_d, wqkv_d, wout_d, y_d):
    from contextlib import ExitStack
    ctx = ExitStack()
    with ctx:
        weights = ctx.enter_context(tc.tile_pool(name="weights", bufs=1))
        qkvp = ctx.enter_context(tc.tile_pool(name="qkv", bufs=1))
        apool = ctx.enter_context(tc.tile_pool(name="attnout", bufs=1))

        wqkv = weights.tile([128, DC, 704], F16, tag="wqkv")
        wout01 = weights.tile([128, D], F16, tag="wout01")
        wout2b = weights.tile([65, D], F16, tag="wout2b")
        qT01 = qkvp.tile([128, N], F16, tag="qT01")
        qT2 = qkvp.tile([128, N], F16, tag="qT2")
        kTz = [qkvp.tile([128, N], F16, tag=f"kTz{h}", name=f"kTz{h}")
               for h in range(3)]
        vAll = qkvp.tile([128, NKC, 3, 128], F16, tag="vAll")
        A01 = apool.tile([128, N], F16, tag="A01")
        A2b = apool.tile([65, N], F16, tag="A2b")

        # one-time zero/one fills of the padded regions (overlap input DMA)
        nc.gpsimd.memset(qT2[64:128, :], 0.0)
        nc.gpsimd.memset(kTz[0][64:128, :], 0.0)
        nc.vector.memset(kTz[1][0:64, :], 0.0)
        nc.gpsimd.memset(kTz[2][64:128, :], 0.0)
        for hh in range(3):  # rank-3 APs only (rank-4 memsets miswrite)
            nc.gpsimd.memset(vAll[:, :, hh, 65:128], 0.0)
            nc.vector.memset(vAll[:, :, hh, 64:65], 1.0)
        nc.gpsimd.memset(A2b[64:65, :], 1.0)

        # ================= phase A: fused k/v projections ================
        # q projections for segments 1-3 are deferred into phase B (emitted
        # as PE filler inside DVE exp windows), so phase A only produces k,
        # v for all segments plus q for segment 0 (needed by qb=0).
        QCOPIES = {0: ((qT01, 0, 128, 0),), 2: ((qT2, 0, 64, 1),)}
        cp = [nc.vector.tensor_copy, nc.scalar.copy]
        xTp = ctx.enter_context(tc.tile_pool(name="xT", bufs=3))
        xtiles = {}

        def emit_qk(xT, seg, ci, nb):
            acc = sps_q[0].tile([128, 512], F32, tag="s", name="qacc")
            c0 = 128 * ci
            col0 = seg * SEG
            for c in range(DC):
                nc.tensor.matmul(acc[:], wqkv[:, c, c0:c0 + 128],
                                 xT[:, c, nb * 512:(nb + 1) * 512],
                                 start=(c == 0), stop=(c == DC - 1))
            cc = col0 + nb * 512
            for dst, lo, hi, eng in (QCOPIES[ci] if ci in (0, 2) else
                                     (((kTz[0], 0, 64, 0), (kTz[1], 64, 128, 1))
                                      if ci == 1 else ((kTz[2], 0, 64, 0),))):
                cp[eng](dst[lo:hi, cc:cc + 512], acc[lo:hi, :])

        with tc.tile_pool(name="vps", bufs=2, space=bass.MemorySpace.PSUM) as vps, \
             tc.tile_pool(name="qkps", bufs=2, space=bass.MemorySpace.PSUM) as qkps:
            sps_q = [qkps]
            nc.sync.dma_start(
                wqkv[:], wqkv_d.ap().rearrange("(c p) m -> p c m", p=128))
            nc.sync.dma_start(wout01[:], wout_d.ap()[0:128, :])
            nc.sync.dma_start(wout2b[:], wout_d.ap()[128:193, :])

            for seg in range(NSEG):
                t0 = seg * SEGC
                col0 = seg * SEG
                xT = xTp.tile([128, DC, SEG], F16, tag="xT")
                xtiles[seg] = xT
                nc.sync.dma_start(
                    xT[:],
                    x_d.ap().rearrange("(c p) n -> p c n", p=128)
                    [:, :, col0:col0 + SEG])
                for ci in (1, 3):
                    for nb in range(SEG // 512):
                        emit_qk(xT, seg, ci, nb)
                for t in range(SEGC):
                    acc = vps.tile([128, 192], F32, tag="vps")
                    for c in range(DC):
                        nc.tensor.matmul(acc[:],
                                         xT[:, c, t * 128:(t + 1) * 128],
                                         wqkv[:, c, 512:704],
                                         start=(c == 0), stop=(c == DC - 1))
                    cp[t % 2](vAll[:, t0 + t, :, 0:64],
                              acc[:].rearrange("p (h d) -> p h d", h=3))
                if seg == 0:
                    for ci in (0, 2):
                        for nb in range(SEG // 512):
                            emit_qk(xT, seg, ci, nb)
        # deferred q-projection work units for phase B's PE-stall windows
        fillers = [(seg, ci, nb) for seg in (1, 2, 3) for ci in (0, 2)
                   for nb in range(SEG // 512)]

        # ===== phase B: software-pipelined flash attention + out proj ====
        steps = [(qb, h, kc)
                 for qb in range(NQC) for h in range(3) for kc in range(NKC)]
        with tc.tile_pool(name="sps", bufs=2, space=bass.MemorySpace.PSUM) as sps, \
             tc.tile_pool(name="ops", bufs=2, space=bass.MemorySpace.PSUM) as ops, \
             tc.tile_pool(name="pp", bufs=4) as pp, \
             tc.tile_pool(name="y1p", bufs=2) as y1p, \
             tc.tile_pool(name="ysbp", bufs=3) as ysbp, \
             tc.tile_pool(name="rp", bufs=2) as rp, \
             tc.tile_pool(name="rbp", bufs=2) as rbp:
            otile = {}
            pending = []

            def emit_S(i):
                qb, h, kc = steps[i]
                s = sps.tile([128, QC], F32, tag="s")
                kt = kTz[h]
                qt = qT01 if h < 2 else qT2
                q0 = qb * QC
                for j in (0, 512):
                    nc.tensor.matmul(s[:, j:j + 512],
                                     kt[:, kc * 128:(kc + 1) * 128],
                                     qt[:, q0 + j:q0 + j + 512],
                                     start=True, stop=True)
                return s

            sps_q[0] = sps  # deferred q-proj accs share the S psum slots

            def is_dve(i):
                kc = steps[i][2]
                return kc % 3 == 2 and kc < 24  # 8 of 32 chunks on the DVE

            def emit_EXP(i, s):
                p = pp.tile([128, QC], F16, tag="p")
                if is_dve(i):
                    y1 = y1p.tile([128, QC], F32, tag="y1")
                    nc.vector._custom_dve(POLY_EXP2, out=y1[:], in0=s[:],
                                          s0=A1, s1=A2, imm2=A3)
                    nc.vector._custom_dve(POW16, out=p[:], in0=y1[:])
                else:
                    nc.scalar.activation(p[:], s[:], EXP, scale=ACT_SCALE)
                return p

            def emit_O(i, p):
                qb, h, kc = steps[i]
                if kc == 0:
                    otile[(qb, h)] = ops.tile([128, QC], F32, tag="o", name="o")
                o = otile[(qb, h)]
                for j in (0, 512):
                    nc.tensor.matmul(o[:, j:j + 512], vAll[:, kc, h, :],
                                     p[:, j:j + 512],
                                     start=(kc == 0), stop=(kc == NKC - 1))

            def finish_head(qb, h):
                # off the DVE critical path: Act copies PSUM out, DVE only
                # does the reciprocal, gpsimd broadcasts + multiplies.
                o = otile[(qb, h)]
                den = rp.tile([1, QC], F32, tag="den", name="den")
                nc.vector.tensor_copy(den[:], o[64:65, :])
                osb = rbp.tile([64, QC], F32, tag="osb", name="osb")
                nc.scalar.copy(osb[:], o[0:64, :])
                rc = rp.tile([1, QC], F32, tag="rc")
                nc.vector.reciprocal_approx_fast(rc[:], den[:])
                rcb = rbp.tile([64, QC], F32, tag="rcb")
                nc.gpsimd.partition_broadcast(rcb[:], rc[:])
                qs = slice(qb * QC, (qb + 1) * QC)
                dst = (A01[0:64, qs] if h == 0 else
                       A01[64:128, qs] if h == 1 else A2b[0:64, qs])
                nc.gpsimd.tensor_mul(dst, osb[:], rcb[:])

            def emit_y(t):
                ts = slice(t * 128, (t + 1) * 128)
                y = sps.tile([128, D], F32, tag="s", name="y")
                for c0, c1 in ((0, 512), (512, 768)):
                    nc.tensor.matmul(y[:, c0:c1], A01[:, ts],
                                     wout01[:, c0:c1], start=True, stop=False)
                    nc.tensor.matmul(y[:, c0:c1], A2b[0:65, ts],
                                     wout2b[:, c0:c1], start=False, stop=True)
                ysb = ysbp.tile([128, D], F32, tag="ysb")
                cp[t % 2](ysb[:], y[:])
                nc.sync.dma_start(y_d.ap()[ts, :], ysb[:])

            # lookahead-2 software pipeline: the PE queue sees S(i+2) (and,
            # on DVE steps, an emit_y burst) before O(i), so the ~2.4us
            # serial DVE exp chain never starves the in-order PE.
            nsteps = len(steps)
            ptile = {0: emit_EXP(0, emit_S(0)), 1: emit_EXP(1, emit_S(1))}
            for i in range(nsteps):
                if i + 2 < nsteps:
                    ptile[i + 2] = emit_EXP(i + 2, emit_S(i + 2))
                if is_dve(i):
                    if fillers:  # deferred q projections fill the DVE window
                        seg, ci, nb = fillers.pop(0)
                        emit_qk(xtiles[seg], seg, ci, nb)
                    elif pending:
                        emit_y(pending.pop(0))
                emit_O(i, ptile.pop(i))
                qb, h, kc = steps[i]
                if kc == NKC - 1:
                    finish_head(qb, h)
                    if h == 2:
                        pending.extend(range(qb * SEGC, (qb + 1) * SEGC))
            for t in pending:
                emit_y(t)


def _get_nc():
    global _nc_cache
    if _nc_cache is None:
        _nc_cache = _build_module()
    return _nc_cache


def kernel(x, W_qkv, W_out, b_out):
    global LAST_RESULT
    x = np.asarray(x, dtype=np.float32)
    W_qkv = np.asarray(W_qkv, dtype=np.float32)
    W_out = np.asarray(W_out, dtype=np.float32)
    b_out = np.asarray(b_out, dtype=np.float32)

    in_maps = []
    for c in range(N_CORES):
        b, j = divmod(c, 4)
        h0 = 3 * j
        q0, k0, v0 = 64 * h0, D + 64 * h0, 2 * D + 64 * h0
        q01 = W_qkv[:, q0:q0 + 128] * ALPHA
        k01 = W_qkv[:, k0:k0 + 128]
        q2 = W_qkv[:, q0 + 128:q0 + 192] * ALPHA
        k2 = W_qkv[:, k0 + 128:k0 + 192]
        v012 = W_qkv[:, v0:v0 + 192]
        wqkv_slice = np.ascontiguousarray(np.concatenate(
            [q01, k01, q2, k2, k2, q2, v012], axis=1).astype(np.float16))
        r0 = 64 * h0
        bias_row = b_out[None, :] if j == 0 else np.zeros((1, D), np.float32)
        wout_slice = np.ascontiguousarray(np.concatenate(
            [W_out[r0:r0 + 192], bias_row], axis=0).astype(np.float16))
        in_maps.append({
            "x": np.ascontiguousarray(x[b].T.astype(np.float16)),
            "wqkv": wqkv_slice,
            "wout": wout_slice,
        })

    nc = _get_nc()
    kwargs = {}
    if TRACE:
        from concourse import bass_utils as _bu
        _bu.upload_artifacts = lambda tmpdir: "local://" + tmpdir
        kwargs["trace"] = True
        if TRACE_ALL_CORES:
            kwargs["trace_cores"] = list(range(N_CORES))
    res = run_bass_kernel_spmd(nc, in_maps, core_ids=list(range(N_CORES)), **kwargs)
    LAST_RESULT = res

    out = np.empty((B, N, D), dtype=np.float32)
    for b in range(B):
        out[b] = (res.results[4 * b + 0]["y"] + res.results[4 * b + 1]["y"]
                  + res.results[4 * b + 2]["y"] + res.results[4 * b + 3]["y"])
    return out
